# revision 24
# baseline (speedup 1.0000x reference)
"""Bilinear pooling + signed-sqrt + L2-norm + classifier, v4.

Math (same identity as v1): with g = sign(feat)*sqrt(|feat|),
    out[b,c] = g_b^T M_c g_b / norm_b + bias_c,   M_c = W[c].reshape(D,D)
Only the symmetric part matters, so each unordered block pair (u,v) of the
16x16 block grid is shipped once as A = M[u,v] + M[v,u]^T (diag: M[v,v]).

Design:
  * W blocks shipped as fp8 E3M4 (1 B/elem): 8.36 MB/core/pass, half of
    bf16. Scaled per (class, column-group) to absmax 14; the scale is
    divided back out on the host output, costing nothing on device.
  * Block->core assignment via a balanced tournament orientation of K16:
    column v takes in-edges from {v-1..v-7 mod 16} (+ {v-8} for v>=8)
    plus the diagonal, giving 8 columns of 9 blocks and 8 of 8. Core k
    owns columns (8+k) [9 blocks] and (k) [8 blocks] -> uniform SPMD
    program: 2 accumulation groups of 9+8 matmuls per class, with all
    per-core variation in the packed data.
  * Group blocks accumulate in PSUM over bi, so the elementwise multiply
    and ones-reduce shrink from 544 to 64 columns per class, batched 8
    classes per 2KB PSUM bank: one tensor_mul + one ones-matmul per bank.
  * W streams in 6 chunks (2/6/8/8/4/2 classes) alternating between the
    two HWDGE queues (sync/scalar): big chunks keep HBM near line rate,
    the small first chunk starts the PE early, the small last chunk
    shortens the drain tail.

Per core, per pass (measured on 8x axon trn2, For_i slope):
  DMA  8.36 MB W(fp8) + 0.26 MB gt/gc   ~24-25 us  <- bound
  PE   510 x (LDW 128col fp8 + MM N=32) ~11 us
  DVE  4 x tensor_mul [128, <=512]       ~3 us
Host: out[b,c] = sum_cores sum_grp lam[core,c,grp]*o[...] / norm_b + bias_c.
"""

import sys

import numpy as np

if "/opt/trn_rl_repo" not in sys.path:
    sys.path.insert(0, "/opt/trn_rl_repo")

import ml_dtypes

import concourse.bass as bass
import concourse.bacc as bacc
import concourse.mybir as mybir
import concourse.tile as tile
from concourse.bass_utils import run_bass_kernel_spmd

B, D, C = 32, 2048, 30
EPS_SQRT = 1e-10
EPS_NORM = 1e-12

N_CORES = 8
P = 128
NB = D // P            # 16 block-columns
NS = 17                # 9 + 8 blocks per core (g slices; t=8/16 are diag)
SMAX = 14.0            # fp8 e3m4 absmax target (max normal 15.5)

# v5 diagonal-block split: a diag block [[Maa,Mab],[Mba,Mbb]] (64-split)
# only needs Maa, Mbb and Bsym = Mab + Mba^T -> 3/4 of a full tile:
#   T1_A [128,64] = [Maa ; (Bsym/2)^T]   x [gA; 2gC] -> rows jA
#   T2_A [64,64]  = Mbb/2 (partits 64:)  x 2gC       -> rows jC
# (grpB mirrored: T1_B = [Bsym'/2 ; M'bb] x [2gA; gC], T2_B = M'aa/2).
# The factor 2 rides in the diag g slices (exact bf16 scaling), so tile
# magnitudes stay at or below regular-block level and the shared group
# scale is unaffected. Both groups' quarter tiles pack into one 64-col
# range (grpA in partitions 64:, grpB in 0:64), saving 64 of 2176
# columns per class: W/core 8.355 -> 8.11 MB.
CCOLS = 15 * P + 3 * 64    # 2112 w columns per class
O_T1A = 8 * P              # grpA 8 full blocks, then T1_A
O_GB = O_T1A + 64          # grpB 7 full blocks
O_T1B = O_GB + 7 * P       # then T1_B
O_T2 = O_T1B + 64          # packed T2 (grpB rows 0:64, grpA rows 64:)

BANKS = [(0, 8), (8, 16), (16, 24), (24, 30)]
GRPS = ((0, 9), (9, 17))
GW = 2 * B             # 64 psum cols per class (2 groups x 32)
# 15 classes per HWDGE queue (sync: 2+8+5, scalar: 6+8+1) so both queues
# carry equal bytes; small first chunk starts the PE early, small last
# chunk shortens the drain tail. Chunks never cross a PSUM-bank boundary.
CHUNKS = [(0, 2), (2, 8), (8, 16), (16, 24), (24, 29), (29, 30)]

_CACHE = {}


def _core_cols(k):
    """(column, [bi list]) for core k's two groups (9 then 8 blocks)."""
    vA = 8 + k
    biA = [(vA - d) % NB for d in range(1, 8)] + [vA - 8, vA]
    vB = k
    biB = [(vB - d) % NB for d in range(1, 8)] + [vB]
    return (vA, biA), (vB, biB)


UNROLL = 8             # passes per For_i iteration (amortizes the barrier)
STAGGERED = False      # staggered semaphore reset (no all-engine barrier)


def _build_bass(repeat=1, loop_n=None, unroll=UNROLL):
    """One SPMD pass (python-unrolled `repeat`), optionally HW-looped.

    With loop_n, the For_i body holds `unroll` passes: For_i emits an
    all-engine barrier + semaphore reset every iteration, which drains the
    DMA pipeline (~4-6 us); unrolling amortizes it. Within the body the
    tile pools carry dependencies pass-to-pass (chunk i waits only on
    chunk i of the previous pass), so passes software-pipeline freely.
    """
    nc = bacc.Bacc(None, target_bir_lowering=False, debug=False)
    w_d = nc.dram_tensor("w", [P, C * CCOLS], mybir.dt.float8e3,
                         kind="ExternalInput")
    # gt (17*32) and gc-rep (8*64) packed in one tensor: one DMA setup cost
    gg_d = nc.dram_tensor("gg", [P, NS * B + 8 * GW], mybir.dt.bfloat16,
                          kind="ExternalInput")
    out_d = nc.dram_tensor("out", [1, C * GW], mybir.dt.float32,
                           kind="ExternalOutput")

    with tile.TileContext(nc) as tc:
        with (
            # bufs=6 = one full pass of chunks: chunk i of pass k+1 only
            # waits on chunk i of pass k, so the DMA queues prefetch a full
            # pass ahead and never drain at the loop boundary.
            tc.tile_pool(name="wpool", bufs=6) as wpool,
            tc.tile_pool(name="const", bufs=1) as cpool,
            tc.tile_pool(name="spool", bufs=3) as spool,
            # Persistent PSUM banks: every pass accumulates into the same
            # four banks (WAW chained on the in-order PE, so no stall), and
            # the post-loop emit reads them without pinning rotating slots.
            tc.tile_pool(name="psA", bufs=1, space=bass.MemorySpace.PSUM) as ppoolA,
            tc.tile_pool(name="psB", bufs=3, space=bass.MemorySpace.PSUM) as ppoolB,
        ):
            # gg rides the scalar HWDGE queue ahead of that queue's W chunks
            gg_sb = cpool.tile([P, NS * B + 8 * GW], mybir.dt.bfloat16)
            nc.scalar.dma_start(gg_sb[:], gg_d[:])
            gt_sb = gg_sb[:, :NS * B]
            gc_sb = gg_sb[:, NS * B:]
            ones_sb = cpool.tile([P, 1], mybir.dt.bfloat16)
            nc.vector.memset(ones_sb[:], 1.0)
            obuf = cpool.tile([1, C * GW], mybir.dt.float32)
            ps_banks = [
                (b0, b1, ppoolA.tile([P, (b1 - b0) * GW], mybir.dt.float32,
                                     name=f"psbank{bi}"))
                for bi, (b0, b1) in enumerate(BANKS)
            ]

            def one_pass():
                # Pure DMA + matmul: every pass computes identical PSUM
                # banks, so emits happen ONCE after the loop on the final
                # pass's banks. The For_i barrier then only drains the last
                # chunk's matmuls, not the emit chain.
                #
                # Issue all chunk DMAs first: each engine's stream is then
                # pure dma_starts, so a chunk's issue waits only on its own
                # buffer (one pass old, long consumed).
                wts = []
                for i, (cs, ce) in enumerate(CHUNKS):
                    cols = (ce - cs) * CCOLS
                    off = cs * CCOLS
                    wt = wpool.tile([P, cols], mybir.dt.float8e3)
                    eng = nc.sync if i % 2 == 0 else nc.scalar
                    eng.dma_start(wt[:], w_d[:, off:off + cols])
                    wts.append(wt)
                for i, (cs, ce) in enumerate(CHUNKS):
                    wt = wts[i]
                    bank = cs // 8
                    b0, b1, ps = ps_banks[bank]
                    for h in range(ce - cs):
                        cc = cs + h - b0
                        base = h * CCOLS
                        pA = cc * GW
                        pB = cc * GW + B
                        # partial-tile diag matmuls sit INSIDE the full-
                        # region start/stop bracket (PSUM accumulate is
                        # per-address; the group checker wants full-region
                        # open/close).
                        for t in range(8):      # grpA full blocks
                            nc.tensor.matmul(
                                ps[:, pA:pA + B],
                                wt[:, base + t * P:base + (t + 1) * P],
                                gt_sb[:, t * B:(t + 1) * B],
                                start=(t == 0), stop=(t == 7),
                            )
                            if t == 0:
                                nc.tensor.matmul(      # T1_A -> rows 0:64
                                    ps[0:64, pA:pA + B],
                                    wt[:, base + O_T1A:base + O_T1A + 64],
                                    gt_sb[:, 8 * B:9 * B],
                                    start=False, stop=False,
                                    skip_group_check=True,
                                )
                                nc.tensor.matmul(      # T2_A -> rows 64:128
                                    ps[64:128, pA:pA + B],
                                    wt[64:128, base + O_T2:base + O_T2 + 64],
                                    gt_sb[64:128, 8 * B:9 * B],
                                    start=False, stop=False,
                                    skip_group_check=True,
                                )
                        for u in range(7):      # grpB full blocks
                            t = 9 + u
                            nc.tensor.matmul(
                                ps[:, pB:pB + B],
                                wt[:, base + O_GB + u * P:base + O_GB + (u + 1) * P],
                                gt_sb[:, t * B:(t + 1) * B],
                                start=(u == 0), stop=(u == 6),
                            )
                            if u == 0:
                                nc.tensor.matmul(      # T1_B -> rows 64:128
                                    ps[64:128, pB:pB + B],
                                    wt[:, base + O_T1B:base + O_T1B + 64],
                                    gt_sb[:, 16 * B:17 * B],
                                    start=False, stop=False,
                                    skip_group_check=True,
                                )
                                nc.tensor.matmul(      # T2_B -> rows 0:64
                                    ps[0:64, pB:pB + B],
                                    wt[0:64, base + O_T2:base + O_T2 + 64],
                                    gt_sb[0:64, 16 * B:17 * B],
                                    start=False, stop=False,
                                    skip_group_check=True,
                                )

            def emit(ps, b0, lo_c, hi_c):
                n = hi_c - lo_c
                lo, hi = (lo_c - b0) * GW, (hi_c - b0) * GW
                v = spool.tile([P, n * GW], mybir.dt.bfloat16)
                nc.vector.tensor_mul(v[:], ps[:, lo:hi], gc_sb[:, lo:hi])
                ps2 = ppoolB.tile([1, n * GW], mybir.dt.float32)
                nc.tensor.matmul(ps2[:], ones_sb[:], v[:],
                                 start=True, stop=True)
                nc.vector.tensor_copy(obuf[:, lo_c * GW:hi_c * GW], ps2[:])

            if loop_n is not None:
                with tc.For_i(0, loop_n, staggered_reset=STAGGERED):
                    for _ in range(unroll):
                        one_pass()
            else:
                for _ in range(repeat):
                    one_pass()
            # Emit the final pass's PSUM banks (identical every pass).
            for b0, b1, ps in ps_banks:
                for lo_c in range(b0, b1, 4):
                    emit(ps, b0, lo_c, min(lo_c + 4, b1))
            nc.sync.dma_start(out_d[:], obuf[:])
    if not nc.is_finalized():
        nc.finalize()
    return nc


def _prep_inputs(feat, W):
    feat = np.asarray(feat, dtype=np.float32)
    W = np.asarray(W, dtype=np.float32)

    g = np.sign(feat) * np.sqrt(np.abs(feat))
    norm = np.sqrt(np.sum(np.abs(feat), axis=1, dtype=np.float64) ** 2
                   + EPS_SQRT * float(D) * float(D))
    norm = np.maximum(norm, EPS_NORM)

    W4 = W.reshape(C, NB, P, NB, P)  # [c, bi, i, bj, j]
    gbf = g.astype(ml_dtypes.bfloat16).astype(np.float32)
    gT = np.ascontiguousarray(gbf.T)  # [D, B]

    in_maps = []
    lams = []
    for k in range(N_CORES):
        (vA, biA), (vB, biB) = _core_cols(k)
        wk = np.empty((C, P, CCOLS), dtype=np.float32)  # [c, i, col]
        # grpA: 8 full (symmetrized) blocks, then diag split
        for t, bi in enumerate(biA[:8]):
            wk[:, :, t * P:(t + 1) * P] = (
                W4[:, bi, :, vA, :] + W4[:, vA, :, bi, :].transpose(0, 2, 1))
        M = W4[:, vA, :, vA, :]
        Bsym = M[:, :64, 64:] + M[:, 64:, :64].transpose(0, 2, 1)  # [c,iA,jC]
        wk[:, :64, O_T1A:O_T1A + 64] = M[:, :64, :64]              # Maa
        wk[:, 64:, O_T1A:O_T1A + 64] = Bsym.transpose(0, 2, 1) / 2
        wk[:, 64:, O_T2:O_T2 + 64] = M[:, 64:, 64:] / 2            # Mbb/2
        # grpB: 7 full blocks, then diag split (mirrored)
        for u, bi in enumerate(biB[:7]):
            wk[:, :, O_GB + u * P:O_GB + (u + 1) * P] = (
                W4[:, bi, :, vB, :] + W4[:, vB, :, bi, :].transpose(0, 2, 1))
        M2 = W4[:, vB, :, vB, :]
        B2sym = M2[:, :64, 64:] + M2[:, 64:, :64].transpose(0, 2, 1)
        wk[:, :64, O_T1B:O_T1B + 64] = B2sym / 2
        wk[:, 64:, O_T1B:O_T1B + 64] = M2[:, 64:, 64:]             # M'bb
        wk[:, :64, O_T2:O_T2 + 64] = M2[:, :64, :64] / 2           # M'aa/2
        # per-(class, group) scales; the packed T2 column range is split
        # by partition: rows 64: belong to grpA, rows :64 to grpB.
        lam = np.empty((C, 2), dtype=np.float32)
        lam[:, 0] = np.maximum(
            np.abs(wk[:, :, :O_GB]).max(axis=(1, 2)),
            np.abs(wk[:, 64:, O_T2:]).max(axis=(1, 2))) / SMAX
        lam[:, 1] = np.maximum(
            np.abs(wk[:, :, O_GB:O_T2]).max(axis=(1, 2)),
            np.abs(wk[:, :64, O_T2:]).max(axis=(1, 2))) / SMAX
        wk[:, :, :O_GB] /= lam[:, 0][:, None, None]
        wk[:, 64:, O_T2:] /= lam[:, 0][:, None, None]
        wk[:, :, O_GB:O_T2] /= lam[:, 1][:, None, None]
        wk[:, :64, O_T2:] /= lam[:, 1][:, None, None]
        lams.append(lam)
        # [c, i, col] -> [i, (c, col)]
        wk8 = (wk.transpose(1, 0, 2).reshape(P, C * CCOLS)
               .astype(ml_dtypes.float8_e3m4))

        gg = np.empty((P, NS * B + 8 * GW), dtype=np.float32)
        t = 0
        for bj, bis in ((vA, biA), (vB, biB)):
            for bi in bis:
                gg[:, t * B:(t + 1) * B] = gT[bi * P:(bi + 1) * P]
                t += 1
        # fold the diag-split factor 2 into the diag g slices (exact in bf16)
        gg[64:, 8 * B:9 * B] *= 2.0      # grpA diag: [gA; 2gC]
        gg[:64, 16 * B:17 * B] *= 2.0    # grpB diag: [2gA; gC]
        base = NS * B
        for grp, bj in enumerate((vA, vB)):
            for cc in range(8):
                o = base + cc * GW + grp * B
                gg[:, o:o + B] = gT[bj * P:(bj + 1) * P]
        in_maps.append({
            "w": np.ascontiguousarray(wk8),
            "gg": gg.astype(ml_dtypes.bfloat16),
        })
    return in_maps, norm, np.stack(lams)  # lams [cores, C, 2]


def _run(inputs, trace=False, repeat=1):
    feat, W, b = inputs["feat"], inputs["W"], inputs["b"]
    assert feat.shape == (B, D) and W.shape == (C, D * D)

    key = ("nc", repeat)
    if key not in _CACHE:
        _CACHE[key] = _build_bass(repeat)
    nc = _CACHE[key]

    in_maps, norm, lams = _prep_inputs(feat, W)
    res = run_bass_kernel_spmd(nc, in_maps, list(range(N_CORES)), trace=trace)
    parts = np.stack([r["out"] for r in res.results]).astype(np.float64)
    parts = parts.reshape(N_CORES, C, 2, B) * lams[:, :, :, None]
    parts = parts.sum(axis=(0, 2)).T  # [B, C]
    out = parts / norm[:, None] + np.asarray(b, dtype=np.float64)[None, :]
    return out.astype(np.float32), res


def kernel(**inputs):
    return _run(inputs)[0]



# revision 29
# speedup vs baseline: 1.1607x; 1.1607x over previous
"""Bilinear pooling + signed-sqrt + L2-norm + classifier, v4.

Math (same identity as v1): with g = sign(feat)*sqrt(|feat|),
    out[b,c] = g_b^T M_c g_b / norm_b + bias_c,   M_c = W[c].reshape(D,D)
Only the symmetric part matters, so each unordered block pair (u,v) of the
16x16 block grid is shipped once as A = M[u,v] + M[v,u]^T (diag: M[v,v]).

Design:
  * W blocks shipped as fp8 E3M4 (1 B/elem): 8.36 MB/core/pass, half of
    bf16. Scaled per (class, column-group) to absmax 14; the scale is
    divided back out on the host output, costing nothing on device.
  * Block->core assignment via a balanced tournament orientation of K16:
    column v takes in-edges from {v-1..v-7 mod 16} (+ {v-8} for v>=8)
    plus the diagonal, giving 8 columns of 9 blocks and 8 of 8. Core k
    owns columns (8+k) [9 blocks] and (k) [8 blocks] -> uniform SPMD
    program: 2 accumulation groups of 9+8 matmuls per class, with all
    per-core variation in the packed data.
  * Group blocks accumulate in PSUM over bi, so the elementwise multiply
    and ones-reduce shrink from 544 to 64 columns per class, batched 8
    classes per 2KB PSUM bank: one tensor_mul + one ones-matmul per bank.
  * W streams in 6 chunks (2/6/8/8/4/2 classes) alternating between the
    two HWDGE queues (sync/scalar): big chunks keep HBM near line rate,
    the small first chunk starts the PE early, the small last chunk
    shortens the drain tail.

Per core, per pass (measured on 8x axon trn2, For_i slope):
  DMA  8.36 MB W(fp8) + 0.26 MB gt/gc   ~24-25 us  <- bound
  PE   510 x (LDW 128col fp8 + MM N=32) ~11 us
  DVE  4 x tensor_mul [128, <=512]       ~3 us
Host: out[b,c] = sum_cores sum_grp lam[core,c,grp]*o[...] / norm_b + bias_c.
"""

import sys

import numpy as np

if "/opt/trn_rl_repo" not in sys.path:
    sys.path.insert(0, "/opt/trn_rl_repo")

import ml_dtypes

import concourse.bass as bass
import concourse.bacc as bacc
import concourse.mybir as mybir
import concourse.tile as tile
from concourse.bass_utils import run_bass_kernel_spmd

B, D, C = 32, 2048, 30
EPS_SQRT = 1e-10
EPS_NORM = 1e-12

N_CORES = 8
P = 128
NB = D // P            # 16 block-columns
NS = 17                # 9 + 8 blocks per core (g slices; t=8/16 are diag)
SMAX = 14.0            # fp8 e3m4 absmax target (max normal 15.5)

# v5 diagonal-block split: a diag block [[Maa,Mab],[Mba,Mbb]] (64-split)
# only needs Maa, Mbb and Bsym = Mab + Mba^T -> 3/4 of a full tile:
#   T1_A [128,64] = [Maa ; (Bsym/2)^T]   x [gA; 2gC] -> rows jA
#   T2_A [64,64]  = Mbb/2 (partits 64:)  x 2gC       -> rows jC
# (grpB mirrored: T1_B = [Bsym'/2 ; M'bb] x [2gA; gC], T2_B = M'aa/2).
# The factor 2 rides in the diag g slices (exact bf16 scaling), so tile
# magnitudes stay at or below regular-block level and the shared group
# scale is unaffected. Both groups' quarter tiles pack into one 64-col
# range (grpA in partitions 64:, grpB in 0:64), saving 64 of 2176
# columns per class: W/core 8.355 -> 8.11 MB.
CCOLS = 15 * P + 3 * 64    # 2112 w columns per class
O_T1A = 8 * P              # grpA 8 full blocks, then T1_A
O_GB = O_T1A + 64          # grpB 7 full blocks
O_T1B = O_GB + 7 * P       # then T1_B
O_T2 = O_T1B + 64          # packed T2 (grpB rows 0:64, grpA rows 64:)

BANKS = [(0, 8), (8, 16), (16, 24), (24, 30)]
GRPS = ((0, 9), (9, 17))
GW = 2 * B             # 64 psum cols per class (2 groups x 32)
# 15 classes per HWDGE queue (sync: 2+8+5, scalar: 6+8+1) so both queues
# carry equal bytes; small first chunk starts the PE early, small last
# chunk shortens the drain tail. Chunks never cross a PSUM-bank boundary.
CHUNKS = [(0, 2), (2, 8), (8, 16), (16, 24), (24, 29), (29, 30)]

_CACHE = {}


def _core_cols(k):
    """(column, [bi list]) for core k's two groups (9 then 8 blocks)."""
    vA = 8 + k
    biA = [(vA - d) % NB for d in range(1, 8)] + [vA - 8, vA]
    vB = k
    biB = [(vB - d) % NB for d in range(1, 8)] + [vB]
    return (vA, biA), (vB, biB)


UNROLL = 8             # passes per For_i iteration (amortizes the barrier)
STAGGERED = False      # staggered semaphore reset (no all-engine barrier)


def _build_bass(repeat=1, loop_n=None, unroll=UNROLL):
    """One SPMD pass (python-unrolled `repeat`), optionally HW-looped.

    With loop_n, the For_i body holds `unroll` passes: For_i emits an
    all-engine barrier + semaphore reset every iteration, which drains the
    DMA pipeline (~4-6 us); unrolling amortizes it. Within the body the
    tile pools carry dependencies pass-to-pass (chunk i waits only on
    chunk i of the previous pass), so passes software-pipeline freely.
    """
    nc = bacc.Bacc(None, target_bir_lowering=False, debug=False)
    w_d = nc.dram_tensor("w", [P, C * CCOLS], mybir.dt.float8e3,
                         kind="ExternalInput")
    # gt (17*32) and gc-rep (8*64) packed in one tensor: one DMA setup cost
    gg_d = nc.dram_tensor("gg", [P, NS * B + 8 * GW], mybir.dt.bfloat16,
                          kind="ExternalInput")
    out_d = nc.dram_tensor("out", [1, C * GW], mybir.dt.float32,
                           kind="ExternalOutput")

    with tile.TileContext(nc) as tc:
        with (
            # bufs=1: For_i_pipelined's PipelineAllocator carves its own
            # N-buffered copies out of this pool (2 full passes of chunks).
            tc.tile_pool(name="wpool", bufs=1) as wpool,
            tc.tile_pool(name="const", bufs=1) as cpool,
            tc.tile_pool(name="spool", bufs=3) as spool,
            # Persistent PSUM banks: every pass accumulates into the same
            # four banks (WAW chained on the in-order PE, so no stall), and
            # the post-loop emit reads them without pinning rotating slots.
            tc.tile_pool(name="psA", bufs=1, space=bass.MemorySpace.PSUM) as ppoolA,
            tc.tile_pool(name="psB", bufs=3, space=bass.MemorySpace.PSUM) as ppoolB,
        ):
            # gg rides the scalar HWDGE queue ahead of that queue's W chunks
            gg_sb = cpool.tile([P, NS * B + 8 * GW], mybir.dt.bfloat16)
            nc.scalar.dma_start(gg_sb[:], gg_d[:])
            gt_sb = gg_sb[:, :NS * B]
            gc_sb = gg_sb[:, NS * B:]
            ones_sb = cpool.tile([P, 1], mybir.dt.bfloat16)
            nc.vector.memset(ones_sb[:], 1.0)
            obuf = cpool.tile([1, C * GW], mybir.dt.float32)
            ps_banks = [
                (b0, b1, ppoolA.tile([P, (b1 - b0) * GW], mybir.dt.float32,
                                     name=f"psbank{bi}"))
                for bi, (b0, b1) in enumerate(BANKS)
            ]

            def load(pipe, iv):
                # Issue all 6 chunk DMAs; each engine's stream is pure
                # dma_starts. Emitted LAST within each pipeline tick, so
                # the body's final loads are issued before the For_i
                # back-edge and their transfers span the barrier.
                wts = []
                for i, (cs, ce) in enumerate(CHUNKS):
                    wt = pipe.intermediate_tile(
                        [P, (ce - cs) * CCOLS], mybir.dt.float8e3,
                        name=f"wt{i}")
                    eng = nc.sync if i % 2 == 0 else nc.scalar
                    eng.dma_start(wt[:], w_d[:, cs * CCOLS:ce * CCOLS])
                    wts.append(wt)
                return tuple(wts)

            def compute(wts):
                # Pure matmul: every pass writes the same persistent PSUM
                # banks, so emits happen ONCE after the loop on the final
                # pass's banks. The For_i barrier then only drains the last
                # chunk's matmuls, not the emit chain.
                for i, (cs, ce) in enumerate(CHUNKS):
                    wt = wts[i]
                    bank = cs // 8
                    b0, b1, ps = ps_banks[bank]
                    for h in range(ce - cs):
                        cc = cs + h - b0
                        base = h * CCOLS
                        pA = cc * GW
                        pB = cc * GW + B
                        # partial-tile diag matmuls sit INSIDE the full-
                        # region start/stop bracket (PSUM accumulate is
                        # per-address; the group checker wants full-region
                        # open/close).
                        for t in range(8):      # grpA full blocks
                            nc.tensor.matmul(
                                ps[:, pA:pA + B],
                                wt[:, base + t * P:base + (t + 1) * P],
                                gt_sb[:, t * B:(t + 1) * B],
                                start=(t == 0), stop=(t == 7),
                            )
                            if t == 0:
                                nc.tensor.matmul(      # T1_A -> rows 0:64
                                    ps[0:64, pA:pA + B],
                                    wt[:, base + O_T1A:base + O_T1A + 64],
                                    gt_sb[:, 8 * B:9 * B],
                                    start=False, stop=False,
                                    skip_group_check=True,
                                )
                                nc.tensor.matmul(      # T2_A -> rows 64:128
                                    ps[64:128, pA:pA + B],
                                    wt[64:128, base + O_T2:base + O_T2 + 64],
                                    gt_sb[64:128, 8 * B:9 * B],
                                    start=False, stop=False,
                                    skip_group_check=True,
                                )
                        for u in range(7):      # grpB full blocks
                            t = 9 + u
                            nc.tensor.matmul(
                                ps[:, pB:pB + B],
                                wt[:, base + O_GB + u * P:base + O_GB + (u + 1) * P],
                                gt_sb[:, t * B:(t + 1) * B],
                                start=(u == 0), stop=(u == 6),
                            )
                            if u == 0:
                                nc.tensor.matmul(      # T1_B -> rows 64:128
                                    ps[64:128, pB:pB + B],
                                    wt[:, base + O_T1B:base + O_T1B + 64],
                                    gt_sb[:, 16 * B:17 * B],
                                    start=False, stop=False,
                                    skip_group_check=True,
                                )
                                nc.tensor.matmul(      # T2_B -> rows 0:64
                                    ps[0:64, pB:pB + B],
                                    wt[0:64, base + O_T2:base + O_T2 + 64],
                                    gt_sb[0:64, 16 * B:17 * B],
                                    start=False, stop=False,
                                    skip_group_check=True,
                                )

            def emit(ps, b0, lo_c, hi_c):
                n = hi_c - lo_c
                lo, hi = (lo_c - b0) * GW, (hi_c - b0) * GW
                v = spool.tile([P, n * GW], mybir.dt.bfloat16)
                nc.vector.tensor_mul(v[:], ps[:, lo:hi], gc_sb[:, lo:hi])
                ps2 = ppoolB.tile([1, n * GW], mybir.dt.float32)
                nc.tensor.matmul(ps2[:], ones_sb[:], v[:],
                                 start=True, stop=True)
                nc.vector.tensor_copy(obuf[:, lo_c * GW:hi_c * GW], ps2[:])

            def comp(pipe, iv, wts):
                compute(wts)

            if loop_n is not None:
                # loop_n = total passes; callers pass k*unroll + 1 so the
                # static remainder is empty and program shape is identical
                # across trip counts (slope timing needs that).
                tc.For_i_pipelined([load, comp], 0, loop_n,
                                   pool=wpool, unroll=unroll,
                                   staged_num_bufs=2)
            else:

                class _OneShot:
                    n_bufs = 1
                    idx_to_use = 0

                    def intermediate_tile(self, shape, dtype, name=None, **kw):
                        return wpool.tile(shape, dtype, name=name or "wt")

                pipe = _OneShot()
                for _ in range(repeat):
                    compute(load(pipe, 0))
            # Emit the final pass's PSUM banks (identical every pass).
            for b0, b1, ps in ps_banks:
                for lo_c in range(b0, b1, 4):
                    emit(ps, b0, lo_c, min(lo_c + 4, b1))
            nc.sync.dma_start(out_d[:], obuf[:])
    if not nc.is_finalized():
        nc.finalize()
    return nc


def _prep_inputs(feat, W):
    feat = np.asarray(feat, dtype=np.float32)
    W = np.asarray(W, dtype=np.float32)

    g = np.sign(feat) * np.sqrt(np.abs(feat))
    norm = np.sqrt(np.sum(np.abs(feat), axis=1, dtype=np.float64) ** 2
                   + EPS_SQRT * float(D) * float(D))
    norm = np.maximum(norm, EPS_NORM)

    W4 = W.reshape(C, NB, P, NB, P)  # [c, bi, i, bj, j]
    gbf = g.astype(ml_dtypes.bfloat16).astype(np.float32)
    gT = np.ascontiguousarray(gbf.T)  # [D, B]

    in_maps = []
    lams = []
    for k in range(N_CORES):
        (vA, biA), (vB, biB) = _core_cols(k)
        wk = np.empty((C, P, CCOLS), dtype=np.float32)  # [c, i, col]
        # grpA: 8 full (symmetrized) blocks, then diag split
        for t, bi in enumerate(biA[:8]):
            wk[:, :, t * P:(t + 1) * P] = (
                W4[:, bi, :, vA, :] + W4[:, vA, :, bi, :].transpose(0, 2, 1))
        M = W4[:, vA, :, vA, :]
        Bsym = M[:, :64, 64:] + M[:, 64:, :64].transpose(0, 2, 1)  # [c,iA,jC]
        wk[:, :64, O_T1A:O_T1A + 64] = M[:, :64, :64]              # Maa
        wk[:, 64:, O_T1A:O_T1A + 64] = Bsym.transpose(0, 2, 1) / 2
        wk[:, 64:, O_T2:O_T2 + 64] = M[:, 64:, 64:] / 2            # Mbb/2
        # grpB: 7 full blocks, then diag split (mirrored)
        for u, bi in enumerate(biB[:7]):
            wk[:, :, O_GB + u * P:O_GB + (u + 1) * P] = (
                W4[:, bi, :, vB, :] + W4[:, vB, :, bi, :].transpose(0, 2, 1))
        M2 = W4[:, vB, :, vB, :]
        B2sym = M2[:, :64, 64:] + M2[:, 64:, :64].transpose(0, 2, 1)
        wk[:, :64, O_T1B:O_T1B + 64] = B2sym / 2
        wk[:, 64:, O_T1B:O_T1B + 64] = M2[:, 64:, 64:]             # M'bb
        wk[:, :64, O_T2:O_T2 + 64] = M2[:, :64, :64] / 2           # M'aa/2
        # per-(class, group) scales; the packed T2 column range is split
        # by partition: rows 64: belong to grpA, rows :64 to grpB.
        lam = np.empty((C, 2), dtype=np.float32)
        lam[:, 0] = np.maximum(
            np.abs(wk[:, :, :O_GB]).max(axis=(1, 2)),
            np.abs(wk[:, 64:, O_T2:]).max(axis=(1, 2))) / SMAX
        lam[:, 1] = np.maximum(
            np.abs(wk[:, :, O_GB:O_T2]).max(axis=(1, 2)),
            np.abs(wk[:, :64, O_T2:]).max(axis=(1, 2))) / SMAX
        wk[:, :, :O_GB] /= lam[:, 0][:, None, None]
        wk[:, 64:, O_T2:] /= lam[:, 0][:, None, None]
        wk[:, :, O_GB:O_T2] /= lam[:, 1][:, None, None]
        wk[:, :64, O_T2:] /= lam[:, 1][:, None, None]
        lams.append(lam)
        # [c, i, col] -> [i, (c, col)]
        wk8 = (wk.transpose(1, 0, 2).reshape(P, C * CCOLS)
               .astype(ml_dtypes.float8_e3m4))

        gg = np.empty((P, NS * B + 8 * GW), dtype=np.float32)
        t = 0
        for bj, bis in ((vA, biA), (vB, biB)):
            for bi in bis:
                gg[:, t * B:(t + 1) * B] = gT[bi * P:(bi + 1) * P]
                t += 1
        # fold the diag-split factor 2 into the diag g slices (exact in bf16)
        gg[64:, 8 * B:9 * B] *= 2.0      # grpA diag: [gA; 2gC]
        gg[:64, 16 * B:17 * B] *= 2.0    # grpB diag: [2gA; gC]
        base = NS * B
        for grp, bj in enumerate((vA, vB)):
            for cc in range(8):
                o = base + cc * GW + grp * B
                gg[:, o:o + B] = gT[bj * P:(bj + 1) * P]
        in_maps.append({
            "w": np.ascontiguousarray(wk8),
            "gg": gg.astype(ml_dtypes.bfloat16),
        })
    return in_maps, norm, np.stack(lams)  # lams [cores, C, 2]


def _run(inputs, trace=False, repeat=1):
    feat, W, b = inputs["feat"], inputs["W"], inputs["b"]
    assert feat.shape == (B, D) and W.shape == (C, D * D)

    key = ("nc", repeat)
    if key not in _CACHE:
        _CACHE[key] = _build_bass(repeat)
    nc = _CACHE[key]

    in_maps, norm, lams = _prep_inputs(feat, W)
    res = run_bass_kernel_spmd(nc, in_maps, list(range(N_CORES)), trace=trace)
    parts = np.stack([r["out"] for r in res.results]).astype(np.float64)
    parts = parts.reshape(N_CORES, C, 2, B) * lams[:, :, :, None]
    parts = parts.sum(axis=(0, 2)).T  # [B, C]
    out = parts / norm[:, None] + np.asarray(b, dtype=np.float64)[None, :]
    return out.astype(np.float32), res


def kernel(**inputs):
    return _run(inputs)[0]

